# revision 52
# baseline (speedup 1.0000x reference)
"""Trainium2 Bass kernel for multi-head causal attention (GQA), 8-way tensor parallel.

Strategy (8 NeuronCores, one chip):
  - Shard heads: core c gets Q heads [c*HQ, (c+1)*HQ) and KV head group c.
    With NH=32, NKV=8, HD=128: HQ=4 Q heads + 1 KV head per core, and the
    GQA repeat_interleave maps exactly those 4 Q heads onto that KV head.
  - Everything on-chip is computed in the "transposed domain": hidden_states
    is fed pre-transposed (X^T: [E, T]) so q^T/k^T come out of the projection
    matmuls with head_dim on partitions, scores come out as S^T [t, s],
    softmax normalization is deferred (no max subtraction needed: |logits|<~7),
    exp'd scores feed P@V directly as the matmul moving operand, and the
    denominator is a ones-vector matmul accumulated alongside.
  - o_proj: per-head AllToAll exchanges attention-output blocks so each core
    ends up with every core's head-h block for its own T/8 token slice; o_proj
    partials accumulate in SBUF (fp32) across heads; the collectives hide
    behind the next head's attention compute. Host concatenates the 8 token
    slices (pure layout, no math).
Compute dtype bf16 (fp32 PSUM accumulation), f32 I/O.
"""

import sys

if "/opt/trn_rl_repo" not in sys.path:
    sys.path.insert(0, "/opt/trn_rl_repo")

import math
import numpy as np

import concourse.bass as bass
import concourse.bacc as bacc
import concourse.tile as tile
import concourse.mybir as mybir
from concourse.bass_utils import run_bass_kernel_spmd

P = 128
N_CORES = 8

FULL_CFG = dict(B=2, S=2048, E=4096, NH=32, NKV=8, HD=128)

CD = mybir.dt.bfloat16   # compute dtype for matmul operands
F32 = mybir.dt.float32


def _derive(cfg):
    B, S, E, NH, NKV, HD = (cfg[k] for k in ("B", "S", "E", "NH", "NKV", "HD"))
    assert HD == P
    d = dict(cfg)
    d["T"] = B * S                    # total tokens (batch-major flatten)
    d["HQ"] = NH // N_CORES           # Q heads per core
    d["JK"] = d["HQ"] * HD            # joined_kv per core
    d["JKF"] = NH * HD                # full joined_kv
    d["EK"] = E // P                  # E k-tiles
    d["TCH"] = 512                    # phase-A token chunk
    d["SCH"] = 512                    # phase-B query chunk
    d["TSLICE"] = d["T"] // N_CORES   # tokens per core after AllToAll
    d["ST"] = S // P                  # key tiles per batch
    assert d["HQ"] * NKV == NH or NH == NKV  # HQ == N_REP: q-heads align with kv head
    assert d["T"] % d["TCH"] == 0 and S % d["SCH"] == 0 and S % d["TCH"] == 0
    assert d["TSLICE"] % P == 0
    return d


def build(cfg=None):
    """Build + compile the 8-core SPMD graph. Returns the Bacc module."""
    c = _derive(cfg or FULL_CFG)
    B, S, E, NH = c["B"], c["S"], c["E"], c["NH"]
    T, HQ, JK, JKF, EK = c["T"], c["HQ"], c["JK"], c["JKF"], c["EK"]
    TCH, SCH, TSLICE, ST = c["TCH"], c["SCH"], c["TSLICE"], c["ST"]
    NCH = T // TCH                   # phase-A chunks
    NSC = T // SCH                   # phase-B query chunks
    SCB = S // SCH                   # query chunks per batch
    NE = max(E // 512, 1)            # o_proj output column chunks
    ECH = min(512, E)
    MT = TSLICE // P                 # output row tiles per core
    inv_sqrt_hd = 1.0 / math.sqrt(c["HD"])
    NOFF = SCH // P                  # distinct diagonal mask offsets

    nc = bacc.Bacc("TRN2", target_bir_lowering=False, debug=False,
                   num_devices=N_CORES)

    xt = nc.dram_tensor("xt", [E, T], F32, kind="ExternalInput").ap()
    wq = nc.dram_tensor("wq", [E, JK], F32, kind="ExternalInput").ap()
    wk = nc.dram_tensor("wk", [E, P], F32, kind="ExternalInput").ap()
    wv = nc.dram_tensor("wv", [E, P], F32, kind="ExternalInput").ap()
    wo = nc.dram_tensor("wo", [JKF, E], F32, kind="ExternalInput").ap()
    masks = nc.dram_tensor("masks", [NOFF, P, SCH], F32, kind="ExternalInput").ap()
    ones = nc.dram_tensor("ones", [P, 1], F32, kind="ExternalInput").ap()
    identity = nc.dram_tensor("identity", [P, P], F32, kind="ExternalInput").ap()
    out = nc.dram_tensor("out", [TSLICE, E], F32, kind="ExternalOutput").ap()

    xt_r = xt.rearrange("(k p) t -> p k t", p=P)
    wq_r = wq.rearrange("(k p) j -> p k j", p=P)
    wk_r = wk.rearrange("(k p) j -> p k j", p=P)
    wv_r = wv.rearrange("(k p) j -> p k j", p=P)
    wo_r = wo.rearrange("(j h p) e -> h p j e", j=N_CORES, h=HQ, p=P)

    with tile.TileContext(nc) as tc:
        with tc.tile_pool(name="const", bufs=1) as const, \
             tc.tile_pool(name="persist", bufs=1) as persist, \
             tc.tile_pool(name="dram", bufs=1, space="DRAM") as dram:

            qT = persist.tile([P, HQ, T], CD)      # q^T: [d, head, token]
            kT = persist.tile([P, T], CD)          # k^T: [d, token]
            vN = persist.tile([P, T // P, P], CD)  # v natural: [t%128, t//128, d]

            a2a_in = []
            a2a_out = []
            for h in range(HQ):
                ain_h = dram.tile([N_CORES * P, TSLICE], CD, tag=f"ain{h}")
                aout_h = dram.tile([N_CORES * P, TSLICE], CD, tag=f"aout{h}")
                a2a_in.append(ain_h)
                a2a_out.append(aout_h)



            # ---- Phase A: q/k/v projections (+ v transpose) ----
            with tc.tile_pool(name="wpool", bufs=1) as wpool, \
                 tc.tile_pool(name="xpool", bufs=2) as xpool, \
                 tc.tile_pool(name="vstage", bufs=2) as vstage, \
                 tc.tile_pool(name="pa", bufs=6, space="PSUM") as pa, \
                 tc.tile_pool(name="pt", bufs=2, space="PSUM") as pt:

                wq_sb = wpool.tile([P, EK, JK], CD, tag="w")
                wk_sb = wpool.tile([P, EK, P], CD, tag="wk")
                wv_sb = wpool.tile([P, EK, P], CD, tag="wv")
                xt_t0 = xpool.tile([P, EK, TCH], CD, tag="x")
                # chunk 0 rides the fast HWDGE f32 path (sync queue) with an
                # on-engine cast: the first ~100us run at the PE's full
                # 2.4 GHz pre-throttle clock, so feeding it early is doubly
                # valuable; weights keep the SWDGE cast path to themselves
                xf32 = wpool.tile([P, EK // 2, TCH], F32, tag="xf32")
                for k in range(EK):
                    nc.gpsimd.dma_start(wq_sb[:, k, :], wq_r[:, k, :])
                    nc.gpsimd.dma_start(wk_sb[:, k, :], wk_r[:, k, :])
                    nc.gpsimd.dma_start(wv_sb[:, k, :], wv_r[:, k, :])
                    if k < EK // 2:
                        nc.sync.dma_start(xf32[:, k, :], xt_r[:, k, 0:TCH])
                        nc.vector.tensor_copy(xt_t0[:, k, :], xf32[:, k, :])
                    elif k == EK // 2:
                        # reuse the staging buffer for the second half
                        for k2 in range(EK // 2, EK):
                            nc.sync.dma_start(xf32[:, k2 - EK // 2, :],
                                              xt_r[:, k2, 0:TCH])
                            nc.vector.tensor_copy(xt_t0[:, k2, :],
                                                  xf32[:, k2 - EK // 2, :])

                mask_sb = const.tile([P, NOFF, SCH], CD)
                nc.gpsimd.dma_start(mask_sb[:], masks.rearrange("o p s -> p o s"))
                ones_sb = const.tile([P, 1], CD)
                nc.gpsimd.dma_start(ones_sb[:], ones)
                ones_row = const.tile([1, P], CD)
                nc.gpsimd.dma_start(ones_row[:], ones.rearrange("p o -> o p"))
                ident = const.tile([P, P], CD)
                nc.gpsimd.dma_start(ident[:], identity)

                for ch in range(NCH):
                    t0 = ch * TCH
                    if ch == 0:
                        xt_t = xt_t0
                    else:
                        xt_t = xpool.tile([P, EK, TCH], CD, tag="x")
                        kg = max(EK // 4, 1)
                        for k0 in range(0, EK, kg):
                            nc.gpsimd.dma_start(
                                xt_t[:, k0:k0 + kg, :],
                                xt_r[:, k0:k0 + kg, t0:t0 + TCH])

                    for h in range(HQ):
                        acc = pa.tile([P, TCH], F32, tag="acc")
                        for k in range(EK):
                            nc.tensor.matmul(acc[:],
                                             wq_sb[:, k, h * P:(h + 1) * P],
                                             xt_t[:, k, :],
                                             start=(k == 0), stop=(k == EK - 1))
                        nc.any.tensor_copy(qT[:, h, t0:t0 + TCH], acc[:])
                    acc = pa.tile([P, TCH], F32, tag="acc")
                    for k in range(EK):
                        nc.tensor.matmul(acc[:], wk_sb[:, k, :], xt_t[:, k, :],
                                         start=(k == 0), stop=(k == EK - 1))
                    nc.any.tensor_copy(kT[:, t0:t0 + TCH], acc[:])
                    acc = pa.tile([P, TCH], F32, tag="acc")
                    for k in range(EK):
                        nc.tensor.matmul(acc[:], wv_sb[:, k, :], xt_t[:, k, :],
                                         start=(k == 0), stop=(k == EK - 1))
                    vt_sb = vstage.tile([P, TCH], CD, tag="vt")
                    nc.any.tensor_copy(vt_sb[:], acc[:])
                    for i in range(TCH // P):
                        ps = pt.tile([P, P], CD, tag="tr")
                        nc.tensor.transpose(ps[:], vt_sb[:, i * P:(i + 1) * P],
                                            ident[:])
                        nc.any.tensor_copy(vN[:, (t0 // P) + i, :], ps[:])

            # ---- Phases B+C, pipelined per head ----
            with tc.tile_pool(name="eb", bufs=5) as eb, \
                 tc.tile_pool(name="ob", bufs=4) as ob, \
                 tc.tile_pool(name="zb", bufs=2) as zb, \
                 tc.tile_pool(name="otp", bufs=2) as otp, \
                 tc.tile_pool(name="wop", bufs=10) as wop, \
                 tc.tile_pool(name="oaccp", bufs=1) as oaccp, \
                 tc.tile_pool(name="ps_s", bufs=3, space="PSUM") as ps_s, \
                 tc.tile_pool(name="ps_o", bufs=2, space="PSUM") as ps_o, \
                 tc.tile_pool(name="ps_zz", bufs=1, space="PSUM") as ps_zz, \
                 tc.tile_pool(name="pc", bufs=2, space="PSUM") as pc:

                out_acc = oaccp.tile([P, MT, E], CD)

                def emit_attention_head(h, pump=None, pump_from=4):
                    # pump: generator emitting o_proj instructions for the
                    # previous head; stepped between attention tiles so the
                    # static PE order interleaves dense o_proj matmuls into
                    # the ACT/DVE-paced attention pipeline
                    def _pump(n):
                        if pump is None:
                            return
                        for _ in range(n):
                            if next(pump, "done") == "done":
                                break
                    for sc in range(NSC):
                        b = sc // SCB
                        jb = sc % SCB
                        s0 = sc * SCH            # global query token offset
                        s0b = jb * SCH           # within-batch offset
                        a = (s0b + SCH) // P     # active key tiles
                        o_ps = ps_o.tile([P, SCH], F32, tag="o")
                        zs_sb = zb.tile([P, SCH], F32, tag="zs")
                        for ti in range(a):
                            tg = b * ST + ti
                            s_ps = ps_s.tile([P, SCH], F32, tag="s")
                            nc.tensor.matmul(s_ps[:], kT[:, tg * P:(tg + 1) * P],
                                             qT[:, h, s0:s0 + SCH],
                                             start=True, stop=True)
                            e_sb = eb.tile([P, SCH], CD, tag="e")
                            nc.scalar.activation(e_sb[:], s_ps[:],
                                                 mybir.ActivationFunctionType.Exp,
                                                 scale=inv_sqrt_hd)
                            off = ti * P - s0b
                            if off >= 0:  # diagonal tile: zero the future keys
                                nc.vector.tensor_mul(e_sb[:], e_sb[:],
                                                     mask_sb[:, off // P, :])
                            nc.tensor.matmul(o_ps[:], vN[:, tg, :], e_sb[:],
                                             start=(ti == 0), stop=(ti == a - 1))
                            # fp32 running sum of exp'd scores (partition-wise)
                            if ti == 0:
                                nc.vector.tensor_copy(zs_sb[:], e_sb[:])
                            else:
                                nc.vector.tensor_add(zs_sb[:], zs_sb[:], e_sb[:])
                            if sc >= pump_from:
                                _pump(10)
                        zs_cd = zb.tile([P, SCH], CD, tag="zscd")
                        nc.scalar.copy(zs_cd[:], zs_sb[:])
                        z_ps = ps_zz.tile([1, SCH], F32, tag="zz")
                        nc.tensor.matmul(z_ps[:], ones_sb[:], zs_cd[:],
                                         start=True, stop=True)

                        zi_sb = zb.tile([1, SCH], F32, tag="zi")
                        nc.vector.reciprocal(zi_sb[:], z_ps[:])
                        zi_cd = zb.tile([1, SCH], CD, tag="zic")
                        nc.vector.tensor_copy(zi_cd[:], zi_sb[:])
                        zbc_ps = ps_zz.tile([P, SCH], F32, tag="zz")
                        nc.tensor.matmul(zbc_ps[:], ones_row[:], zi_cd[:],
                                         start=True, stop=True)
                        zbc_sb = zb.tile([P, SCH], CD, tag="zbc_sb")
                        nc.scalar.copy(zbc_sb[:], zbc_ps[:])
                        o_sb = ob.tile([P, SCH], CD, tag="osb")
                        nc.vector.tensor_mul(o_sb[:], o_ps[:], zbc_sb[:])
                        for js in range(max(SCH // TSLICE, 1)):
                            blk = (s0 + js * TSLICE) // TSLICE
                            col0 = (s0 + js * TSLICE) % TSLICE
                            w = min(TSLICE, SCH)
                            nc.sync.dma_start(
                                a2a_in[h][blk * P:(blk + 1) * P, col0:col0 + w],
                                o_sb[:, js * w:(js + 1) * w])

                    nc.gpsimd.collective_compute(
                        "AllToAll", mybir.AluOpType.bypass,
                        ins=[a2a_in[h].opt()], outs=[a2a_out[h].opt()],
                        replica_groups=[list(range(N_CORES))])

                NQ = max(min(8, NE), 1)
                WN = NE // NQ            # n-chunks per quarter

                def emit_oproj_loads(h):
                    # weight loads emitted at B-section starts so they stream
                    # well before the o_proj matmuls get pumped
                    wons = []
                    for q in range(NQ):
                        won_q = wop.tile([P, N_CORES, WN * ECH], CD, tag="wo")
                        for j0 in range(0, N_CORES, N_CORES // 2):
                            nc.gpsimd.dma_start(
                                won_q[:, j0:j0 + N_CORES // 2, :],
                                wo_r[h][:, j0:j0 + N_CORES // 2,
                                        q * WN * ECH:(q + 1) * WN * ECH])
                        wons.append(won_q)
                    return wons

                def emit_oproj_head(h, wons):
                    ot_h = otp.tile([P, N_CORES, TSLICE], CD, tag="ot")
                    nc.sync.dma_start(
                        ot_h[:], a2a_out[h][:].rearrange("(j p) t -> p j t", p=P))
                    for q in range(NQ):
                        won_q = wons[q]
                        for m in range(MT):
                            for nn in range(WN):
                                n = q * WN + nn
                                acc_c = pc.tile([P, ECH], F32, tag="c")
                                for jj in range(N_CORES):
                                    nc.tensor.matmul(
                                        acc_c[:],
                                        ot_h[:, jj, m * P:(m + 1) * P],
                                        won_q[:, jj, nn * ECH:(nn + 1) * ECH],
                                        start=(jj == 0), stop=(jj == N_CORES - 1))
                                    yield
                                if h == 0:
                                    nc.vector.tensor_copy(
                                        out_acc[:, m, n * ECH:(n + 1) * ECH],
                                        acc_c[:])
                                else:
                                    nc.vector.tensor_add(
                                        out_acc[:, m, n * ECH:(n + 1) * ECH],
                                        out_acc[:, m, n * ECH:(n + 1) * ECH],
                                        acc_c[:])
                                yield

                def drain(gen):
                    if gen is not None:
                        for _ in gen:
                            pass

                # fine-grained software pipeline: while B(h) runs its
                # ACT/DVE-paced attention, the static PE order is peppered
                # with C(h-1)'s dense o_proj matmuls
                wons = emit_oproj_loads(0)
                emit_attention_head(0)
                for h in range(1, HQ):
                    gen = emit_oproj_head(h - 1, wons)
                    wons = emit_oproj_loads(h)
                    emit_attention_head(h, pump=gen, pump_from=6 if h == 1 else 4)
                    drain(gen)
                drain(emit_oproj_head(HQ - 1, wons))

                for m in range(MT):
                    nc.gpsimd.dma_start(out[m * P:(m + 1) * P, :], out_acc[:, m, :])

    nc.compile()
    return nc, c


def _make_masks(cfg):
    c = _derive(cfg)
    SCH = c["SCH"]
    NOFF = SCH // P
    m = np.zeros((NOFF, P, SCH), np.float32)
    for o in range(NOFF):
        for p in range(P):
            lo = p + o * P
            if lo < SCH:
                m[o, p, lo:] = 1.0
    return m


def make_in_maps(cfg, hidden_states, Wq, Wk, Wv, Wo):
    c = _derive(cfg)
    B, S, E, NH, HQ = c["B"], c["S"], c["E"], c["NH"], c["HQ"]
    T = c["T"]
    xt = np.ascontiguousarray(
        np.asarray(hidden_states, np.float32).reshape(T, E).T)
    Wq = np.asarray(Wq, np.float32)
    Wk = np.asarray(Wk, np.float32)
    Wv = np.asarray(Wv, np.float32)
    wo = np.ascontiguousarray(np.asarray(Wo, np.float32))
    masks = _make_masks(cfg)
    ones = np.ones((P, 1), np.float32)
    ident = np.eye(P, dtype=np.float32)
    nkv_per = max(c["NKV"] // N_CORES, 1)
    in_maps = []
    for cidx in range(N_CORES):
        in_maps.append({
            "xt": xt,
            "wq": np.ascontiguousarray(
                Wq[:, cidx * HQ:(cidx + 1) * HQ, :].reshape(E, HQ * P)),
            "wk": np.ascontiguousarray(Wk[:, cidx * nkv_per, :]),
            "wv": np.ascontiguousarray(Wv[:, cidx * nkv_per, :]),
            "wo": wo,
            "masks": masks,
            "ones": ones,
            "identity": ident,
        })
    return in_maps


_CACHE = {}


def _get_built(key, cfg):
    if key not in _CACHE:
        _CACHE[key] = build(cfg)
    return _CACHE[key]


def kernel(hidden_states, Wq, Wk, Wv, Wo):
    cfg = FULL_CFG
    nc, c = _get_built("full", cfg)
    in_maps = make_in_maps(cfg, hidden_states, Wq, Wk, Wv, Wo)
    res = run_bass_kernel_spmd(nc, in_maps, core_ids=list(range(N_CORES)))
    outs = [res.results[i]["out"] for i in range(N_CORES)]
    full = np.concatenate(outs, axis=0)
    return full.reshape(c["B"], c["S"], c["E"]).astype(np.float32)
